# revision 7
# baseline (speedup 1.0000x reference)
"""GCN-Tox21 GNN message-passing kernel for 8 Trainium2 NeuronCores — fp8 rev.

Strategy (graph/edge parallelism), evolved from the bf16 baseline:
  - Edges sorted by destination; core k owns dst-node range [k*NPC,(k+1)*NPC).
  - Node features h live replicated in DRAM as fp8(e4m3) pairs packed in
    uint16 with the feature permutation pi = [0,128,1,129,...]: u16 unit p of
    a row = (feat p, feat p+128). dma_gather at u16 granularity then lands
    per-edge features as byte-interleaved (A,B) pairs — exactly the
    DoubleRowSwInterleave weight layout, so the src-half of the message MLP
    is ONE K=256 fp8 matmul per 128-edge subtile (gather positions are
    reversed per subtile to match SwInterleave's reversed column order).
  - m1 (528->F_mid) per subtile in PSUM f32:
      SwInterleave(gathered h_src, w1s) + DoubleRow([ST ; eT+ones] pairs,
      [Q_window ; w1e+b1]), where Q = h_win @ w1d.T is computed once per
      128-dst-node window (DoubleRow) and expanded per edge via the one-hot
      ST plane. relu -> fp8, split across Act and DVE engines.
  - Segment-MEAN directly in transposed orientation: the one-hot S carries
    invc (=1/cnt) values instead of 1s, and the seg matmul is
    out[F_mid_chunk, nodes] += m1sb.T @ S (DoubleRow over subtile pairs),
    accumulating the whole window in one PSUM bank. No transposes needed.
  - Per window: m2 in BOTH orientations from the fp8 seg result S1T:
      node-major h (for the DRAM gather copy) and feature-major hT (for the
      next layer's Q), each a couple of DoubleRow matmuls; BN/bias folded
      (cnt*invc == 1 makes the bias a constant row: activation bias column
      in the transposed orientation, K=1 ones-row matmul in node-major).
  - AllGather (uint16 fp8-pairs) rebuilds replicated h after each layer,
    including h0 (computed own-stripe only).
  - Pooling/FC unchanged from baseline (bf16): batch-sorted graphs, one-hot
    pooling matmul, host applies the final sigmoid.
"""

import numpy as np
import ml_dtypes

import concourse.bacc as bacc
import concourse.tile as tile
from concourse import mybir, bass_utils
from concourse.masks import make_identity

F8 = mybir.dt.float8e4
BF16 = mybir.dt.bfloat16
F32 = mybir.dt.float32
U16 = mybir.dt.uint16
I16 = mybir.dt.int16
NPF8 = ml_dtypes.float8_e4m3fn
RELU = mybir.ActivationFunctionType.Relu
DR = mybir.MatmulPerfMode.DoubleRow
DRSW = mybir.MatmulPerfMode.DoubleRowSwInterleave

N_CORES = 8
BN_EPS = 1e-5
G_REAL = 512
F_NODE, F_EDGE, H, EH = 32, 8, 256, 16
OUT_DIMS = (256, 256, 128)
EG = 512  # edges per gather batch

P256 = np.empty(256, np.int64)
P256[0::2] = np.arange(128)
P256[1::2] = np.arange(128) + 128


def _bf(a):
    return np.ascontiguousarray(a.astype(ml_dtypes.bfloat16))


def _f8(a):
    return np.ascontiguousarray(a.astype(NPF8))


def _f32(a):
    return np.ascontiguousarray(a.astype(np.float32))


def _wrap_idx(idx):
    """int16 index layout for dma_gather: index i at [i % 16, i // 16],
    replicated across the 8 partition groups."""
    assert len(idx) % 16 == 0
    w = idx.astype(np.int16).reshape(-1, 16).T
    return np.ascontiguousarray(np.tile(w, (8, 1)))


class Plan:
    """Host-side preprocessing: sharding layout + per-core input tensors."""

    def __init__(self, inputs, G):
        x = np.asarray(inputs["x"]).astype(np.float32)
        N = x.shape[0]
        self.N, self.G = N, G
        self.N_pad = ((N + N_CORES * 128 - 1) // (N_CORES * 128)) * (N_CORES * 128)
        self.NPC = self.N_pad // N_CORES
        self.W = self.NPC // 128
        assert G % N_CORES == 0
        self.GPC = G // N_CORES

        edge_index = np.asarray(inputs["edge_index"]).astype(np.int64)
        src, dst = edge_index[0].astype(np.int32), edge_index[1].astype(np.int32)
        batch = np.asarray(inputs["batch"]).astype(np.int32)
        edge_attr = np.asarray(inputs["edge_attr"]).astype(np.float32)

        order = np.argsort(dst, kind="stable")
        s_dst, s_src = dst[order], src[order]
        s_ea = edge_attr[order]

        bounds = np.searchsorted(s_dst, np.arange(0, self.N_pad + 1, 128), "left")
        cnt_w = (bounds[1:] - bounds[:-1]).reshape(N_CORES, self.W)
        T_w = np.maximum(1, -(-cnt_w.max(axis=0) // 128))
        while T_w.sum() % (EG // 128) != 0:
            T_w[-1] += 1
        self.T_w = [int(t) for t in T_w]
        self.T_tot = int(T_w.sum())
        self.ET = self.T_tot * 128

        cnt = np.bincount(dst, minlength=self.N_pad).astype(np.float32)
        invc_full = 1.0 / np.maximum(cnt, 1.0)
        gcnt = np.bincount(batch, minlength=G).astype(np.float32)
        ginv_full = 1.0 / np.maximum(gcnt, 1.0)

        lo_k = [int(np.searchsorted(batch, k * self.GPC, "left")) for k in range(N_CORES)]
        hi_k = [int(np.searchsorted(batch, (k + 1) * self.GPC, "left")) for k in range(N_CORES)]
        self.TP = max(1, max(-(-(h - l) // 128) for l, h in zip(lo_k, hi_k)))
        self.NPOOL = self.TP * 128

        self.per_core = []
        for k in range(N_CORES):
            d = {}
            gi_src = np.zeros(self.ET, np.int32)
            ea_pad = np.zeros((self.ET, F_EDGE), np.float32)
            # S with invc values (seg-mean folded in), [128 edge, T, 128 node]
            Sv = np.zeros((128, self.T_tot, 128), np.float32)
            pos = 0
            for w in range(self.W):
                base = k * self.NPC + w * 128
                lo = np.searchsorted(s_dst, base, "left")
                hi = np.searchsorted(s_dst, base + 128, "left")
                n = hi - lo
                sl = slice(pos, pos + n)
                gi_src[sl] = s_src[lo:hi]
                ea_pad[sl] = s_ea[lo:hi]
                loc = (s_dst[lo:hi] - base).astype(np.int64)
                e_ids = np.arange(pos, pos + n)
                Sv[e_ids % 128, e_ids // 128, loc] = invc_full[base + loc]
                pos += self.T_w[w] * 128
            assert pos == self.ET

            # reverse gather positions within each 128-subtile (SwInterleave)
            gi_rev = gi_src.reshape(-1, 128)[:, ::-1].reshape(-1)
            d["gidx_src"] = _wrap_idx(gi_rev)
            d["S_dr"] = _f8(Sv)
            # C: [128, T, 2, 128]: plane0 = ST (one-hot transposed, 1s),
            # plane1 = rows 0..15 eT (device-written), row 16 ones, rest 0
            C = np.zeros((128, self.T_tot, 2, 128), np.float32)
            ST = (Sv > 0).astype(np.float32)  # [e, t, n] -> transpose to [n, t, e]
            C[:, :, 0, :] = ST.transpose(2, 1, 0)
            C[16, :, 1, :] = 1.0
            d["C_in"] = _f8(C)
            eaT = np.concatenate([ea_pad.T, np.ones((1, self.ET), np.float32)], 0)
            d["eaT"] = _bf(eaT)
            lo, hi = lo_k[k], hi_k[k]
            pidx = np.zeros(self.NPOOL, np.int32)
            pidx[: hi - lo] = np.arange(lo, hi)
            d["pool_idx"] = _wrap_idx(pidx)
            S2 = np.zeros((128, self.TP * self.GPC), np.float32)
            pb = batch[lo:hi] - k * self.GPC
            e_ids = np.arange(hi - lo)
            S2[e_ids % 128, (e_ids // 128) * self.GPC + pb] = 1.0
            d["S2"] = _bf(S2)
            d["ginv"] = _f32(ginv_full[k * self.GPC:(k + 1) * self.GPC].reshape(self.GPC, 1))
            x_own = np.zeros((self.NPC, F_NODE), np.float32)
            n0, n1 = k * self.NPC, min(N, (k + 1) * self.NPC)
            if n1 > n0:
                x_own[: n1 - n0] = x[n0:n1]
            d["xT_own"] = _bf(np.concatenate(
                [x_own.T, np.ones((1, self.NPC), np.float32)], 0))
            self.per_core.append(d)

        sh = {}
        ne_w, ne_b = _f32(inputs["ne_w"]), _f32(inputs["ne_b"])
        ne_wT = np.concatenate([ne_w.T, ne_b[None, :]], 0)  # [33, 256]
        sh["ne_wT"] = _bf(ne_wT[:, P256])  # pi-permuted output cols
        ee_w, ee_b = _f32(inputs["ee_w"]), _f32(inputs["ee_b"])
        sh["ee_wT"] = _bf(np.concatenate([ee_w.T, ee_b[None, :]], 0))

        in_dim = H
        self.layer_dims = []
        for i, out_dim in enumerate(OUT_DIMS):
            w1 = _f32(inputs[f"c{i}_w1"]); b1 = _f32(inputs[f"c{i}_b1"])
            w2 = _f32(inputs[f"c{i}_w2"]); b2 = _f32(inputs[f"c{i}_b2"])
            g = _f32(inputs[f"bn{i}_g"]); bb = _f32(inputs[f"bn{i}_b"])
            rm = _f32(inputs[f"bn{i}_m"]); rv = _f32(inputs[f"bn{i}_v"])
            A = g / np.sqrt(rv + BN_EPS)
            F_mid = 2 * out_dim
            w1T = w1.T  # [2*in+EH, F_mid]
            w1d, w1s = w1T[:in_dim], w1T[in_dim:2 * in_dim]
            w1e = w1T[2 * in_dim:]
            # src planes: [p, i, n] = w1s[p + 128*i, n]
            sh[f"w1s_{i}"] = _f8(w1s.reshape(2, 128, F_mid).transpose(1, 0, 2))
            # dst planes follow stored (pi) order: [r, i, n] = w1d[P256[i*128+r], n]
            wd = np.ascontiguousarray(
                w1d[P256].reshape(2, 128, F_mid).transpose(1, 0, 2))
            sh[f"w1d_{i}"] = _f8(wd)
            # edge + bias plane: rows 0..15 w1e, row 16 b1, rest zero
            we = np.zeros((128, F_mid), np.float32)
            we[:EH] = w1e
            we[EH] = b1
            sh[f"w1e_{i}"] = _f8(we)
            # w2A with pi-permuted out cols (except last layer), as K-chunk dim
            w2A = (w2 * A[:, None]).T  # [F_mid, F_out]
            pout = P256 if i < 2 else np.arange(out_dim)
            w2p = w2A[:, pout]
            sh[f"w2_{i}"] = _f8(np.ascontiguousarray(
                w2p.reshape(F_mid // 128, 128, out_dim).transpose(1, 0, 2)))
            cb = bb - rm * A + b2 * A
            cbp = cb[pout]
            sh[f"cbrow_{i}"] = _bf(cbp[None, :])
            if i < 2:
                sh[f"cbcol_{i}"] = _f32(cbp.reshape(2, 128).T)  # [128, 2]
            self.layer_dims.append((in_dim, F_mid, out_dim))
            in_dim = out_dim

        fc_w, fc_b = _f32(inputs["fc_w"]), _f32(inputs["fc_b"])
        self.F_FC = fc_w.shape[0]
        sh["fc_wT"] = _bf(fc_w.T)
        sh["fcb_bc"] = _f32(np.tile(fc_b[None, :], (self.GPC, 1)))
        sh["onesrow"] = _bf(np.ones((1, 128), np.float32))
        self.shared = sh

    def in_maps(self):
        out = []
        for k in range(N_CORES):
            m = {**self.shared, **self.per_core[k]}
            out.append({k2: (v.view(np.uint8) if v.dtype == NPF8 else v)
                        for k2, v in m.items()})
        return out


def build_program(plan: Plan, n_cores=N_CORES, repeats=1,
                  skip_gather=False, skip_compute=False,
                  debug_no_collective=False, debug_stage=9):
    nc = bacc.Bacc("TRN2", target_bir_lowering=False, debug=False,
                   num_devices=n_cores, num_swdge_queues=4)

    ET, T_w, W, NPC, TP, GPC = plan.ET, plan.T_w, plan.W, plan.NPC, plan.TP, plan.GPC
    N_pad, NPOOL, F_FC = plan.N_pad, plan.NPOOL, plan.F_FC
    T_tot = plan.T_tot

    sample = {k: np.asarray(v) for k, v in plan.in_maps()[0].items()}
    # declare inputs with proper bass dtypes (fp8 arrives as uint8 bytes)
    F8_NAMES = {"S_dr", "C_in"} | {f"w1s_{i}" for i in range(3)} \
        | {f"w1d_{i}" for i in range(3)} | {f"w1e_{i}" for i in range(3)} \
        | {f"w2_{i}" for i in range(3)}
    t_in = {}
    for name, arr in sample.items():
        dt = F8 if name in F8_NAMES else mybir.dt.from_np(arr.dtype)
        t_in[name] = nc.dram_tensor(name, list(arr.shape), dt, kind="ExternalInput")
    out_part = nc.dram_tensor("out_part", [GPC, F_FC], F32, kind="ExternalOutput")

    # subtile -> window mapping (static)
    sub_window, sub_tw = [], []
    for w in range(W):
        for t in range(T_w[w]):
            sub_window.append(w)
            sub_tw.append(t)

    with tile.TileContext(nc) as tc:
        with (
            tc.tile_pool(name="const", bufs=1) as cpool,
            tc.tile_pool(name="sbuf", bufs=2) as spool,
            tc.tile_pool(name="gath", bufs=6) as gpool,
            tc.tile_pool(name="m1sb", bufs=6) as m1pool,
            tc.tile_pool(name="psum", bufs=2, space="PSUM") as ppool,
            tc.tile_pool(name="dram", bufs=1, space="DRAM") as dpool,
        ):
            def _body():
                # ---------- resident constants ----------
                def load_const(name, dtype=None, tag=None):
                    arr = sample[name]
                    dt = dtype or (F8 if name in F8_NAMES
                                   else mybir.dt.from_np(arr.dtype))
                    shape = list(arr.shape)
                    t = cpool.tile(shape, dt, tag=tag or name)
                    nc.sync.dma_start(out=t[:], in_=t_in[name][:])
                    return t

                ne_wT_t = load_const("ne_wT")
                xT_own_t = load_const("xT_own")
                ee_wT_t = load_const("ee_wT")
                gidx_src = load_const("gidx_src")
                w1s_t, w1d_t, w1e_t, w2_t, cbrow_t, cbcol_t = [], [], [], [], [], []
                for i in range(3):
                    w1s_t.append(load_const(f"w1s_{i}"))
                    w1d_t.append(load_const(f"w1d_{i}"))
                    w1e_t.append(load_const(f"w1e_{i}"))
                    w2_t.append(load_const(f"w2_{i}"))
                    cbrow_t.append(load_const(f"cbrow_{i}"))
                    cbcol_t.append(load_const(f"cbcol_{i}") if i < 2 else None)
                # big tensors chunked so early slices arrive fast
                arr = sample["S_dr"]
                S_t = cpool.tile(list(arr.shape), F8, tag="S_dr", name="S_t")
                arr = sample["C_in"]
                C_t = cpool.tile(list(arr.shape), F8, tag="C_in", name="C_t")
                TQ = T_tot // 4
                for ci in range(4):
                    sl = slice(ci * TQ, (ci + 1) * TQ if ci < 3 else T_tot)
                    nc.scalar.dma_start(out=C_t[:, sl, :, :],
                                        in_=t_in["C_in"][:, sl, :, :])
                    nc.scalar.dma_start(out=S_t[:, sl, :],
                                        in_=t_in["S_dr"][:, sl, :])
                pool_idx = load_const("pool_idx")
                S2_t = load_const("S2")
                ginv_t = load_const("ginv")
                fc_wT_t = load_const("fc_wT")
                fcb_t = load_const("fcb_bc")
                ones_t = load_const("onesrow")
                # hT[li]: [128, W, 2, 128] f8: plane i rows r = stored col i*128+r
                hT_t = [cpool.tile([128, W, 2, 128], F8, tag=f"hT{i}",
                                   name=f"hT{i}")
                        for i in range(3)]
                # Rall: per-window rhs of the expansion DoubleRow [Q_w ; w1e+b1],
                # shared across layers (WAR deps serialize layer reuse)
                Rall = cpool.tile([128, W, 2, 512], F8, tag="Rall", name="Rall")

                # ---------- DRAM buffers ----------
                h_full = [dpool.tile([N_pad, 128], U16, tag=f"h{i}",
                                     name=f"h_full{i}", addr_space="Shared")
                          for i in range(3)]
                h_full3 = dpool.tile([N_pad, 128], BF16, tag="h3", name="h_full3",
                                     addr_space="Shared")
                h_own = [dpool.tile([NPC, 128], U16, tag=f"hown{i}",
                                    name=f"h_own{i}") for i in range(3)]
                h_own3 = dpool.tile([NPC, 128], BF16, tag="hown3", name="h_own3")

                def allgather(src, dst):
                    if debug_no_collective:
                        cp = spool.tile([128, 128], src.dtype, tag="dbgcp")
                        nc.sync.dma_start(out=cp[:], in_=src[0:128, :])
                        nc.sync.dma_start(out=dst[0:128, :], in_=cp[:])
                    else:
                        nc.gpsimd.collective_compute(
                            "AllGather", mybir.AluOpType.bypass,
                            ins=[src.opt()], outs=[dst.opt()],
                            replica_groups=[list(range(n_cores))])

                # ---------- stage A: own-stripe h0 (both orientations) ----------
                for w in range(W):
                    xsl = xT_own_t[:, w * 128:(w + 1) * 128]
                    ps = ppool.tile([128, 4, 128], F32, tag="m1", bufs=3)
                    nc.tensor.matmul(out=ps[:, 0:2, :], lhsT=xsl, rhs=ne_wT_t[:],
                                     start=True, stop=True)
                    h0sb = spool.tile([128, H], F8, tag="h0sb")
                    nc.any.tensor_scalar(out=h0sb[:], in0=ps[:, 0:2, :],
                                         scalar1=0.0, scalar2=None,
                                         op0=mybir.AluOpType.max)
                    nc.sync.dma_start(
                        out=h_own[0][w * 128:(w + 1) * 128, :].bitcast(F8),
                        in_=h0sb[:])
                    for ho in range(2):
                        pt = ppool.tile([128, 128], F32, tag="tp", bufs=1)
                        nc.tensor.matmul(
                            out=pt[:], lhsT=ne_wT_t[:, ho * 128:(ho + 1) * 128],
                            rhs=xsl, start=True, stop=True)
                        nc.any.tensor_scalar(out=hT_t[0][:, w, ho, :],
                                             in0=pt[:], scalar1=0.0,
                                             scalar2=None,
                                             op0=mybir.AluOpType.max)
                allgather(h_own[0], h_full[0])

                # ---------- stage A2: eT -> C plane1 rows 0..15 ----------
                if debug_stage < 2:
                    return
                ea_t = None
                for g0 in range(ET // 512):
                    if g0 % 4 == 0:
                        ea_t = spool.tile([F_EDGE + 1, 2048], BF16, tag="eaT")
                        nc.scalar.dma_start(
                            out=ea_t[:, 0:min(2048, ET - g0 * 512)],
                            in_=t_in["eaT"][:, g0 * 512:
                                            min((g0 + 4) * 512, ET)])
                    c0 = (g0 % 4) * 512
                    ps = ppool.tile([128, 4, 128], F32, tag="m1", bufs=3)
                    nc.tensor.matmul(out=ps[0:EH, :, :], lhsT=ee_wT_t[:],
                                     rhs=ea_t[:, c0:c0 + 512], start=True,
                                     stop=True)
                    nc.any.tensor_scalar(
                        out=C_t[0:EH, 4 * g0:4 * g0 + 4, 1, :],
                        in0=ps[0:EH, :, :], scalar1=0.0, scalar2=None,
                        op0=mybir.AluOpType.max)

                # ---------- conv layers ----------
                if debug_stage < 3:
                    return
                for li, (F_in, F_mid, F_out) in enumerate(plan.layer_dims):
                    h_in = h_full[li]
                    KM = F_mid // 128
                    gs = None
                    node_ps = None
                    m1sb = None
                    first_seg = True
                    if not skip_compute:
                        for w in range(W):
                            nc.any.tensor_copy(out=Rall[:, w, 1, 0:F_mid],
                                               in_=w1e_t[li][:])
                            qp = ppool.tile([128, 4, 128], F32, tag="qt",
                                            bufs=1)
                            nc.tensor.matmul(out=qp[:, 0:KM, :],
                                             lhsT=hT_t[li][:, w, :, :],
                                             rhs=w1d_t[li][:], start=True,
                                             stop=True, perf_mode=DR,
                                             skip_group_check=True)
                            nc.any.tensor_copy(out=Rall[:, w, 0, 0:F_mid],
                                               in_=qp[:, 0:KM, :])
                    for t in range(T_tot):
                        w = sub_window[t]
                        tw = sub_tw[t]
                        last = tw == T_w[w] - 1
                        if t % 4 == 0:
                            gs = gpool.tile([128, 1, EG], U16, tag="gs")
                            if not skip_gather:
                                nc.gpsimd.dma_gather(
                                    gs[:], h_in[:, :],
                                    gidx_src[:, (t * 128) // 16:(t * 128 + EG) // 16],
                                    EG, EG, 128, transpose=True,
                                    queue_num=(t // 4) % 4)
                            else:
                                nc.vector.memset(gs[:], 0)
                        if skip_compute:
                            continue
                        if tw == 0:
                            node_ps = ppool.tile([128, 4, 128], F32, tag="node")
                            first_seg = True
                        # m1 = src(SwInterleave) + [ST;eT] x [Q;w1e] (DoubleRow)
                        m1c = ppool.tile([128, 4, 128], F32, tag="m1", bufs=3)
                        m1p = m1c[:, 0:KM, :]
                        s4 = t % 4
                        nc.tensor.matmul(
                            out=m1p,
                            lhsT=gs[:, 0, s4 * 128:(s4 + 1) * 128].bitcast(F8),
                            rhs=w1s_t[li][:], start=True, stop=False,
                            perf_mode=DRSW, skip_group_check=True)
                        nc.tensor.matmul(
                            out=m1p, lhsT=C_t[:, t, :, :],
                            rhs=Rall[:, w, :, 0:F_mid],
                            start=False, stop=True, perf_mode=DR,
                            skip_group_check=True)
                        par = tw % 2
                        if par == 0:
                            m1sb = m1pool.tile([128, 2, F_mid], F8, tag="m1sb")
                        nc.any.tensor_scalar(
                            out=m1sb[:, par, :], in0=m1c[:, 0:KM, :],
                            scalar1=0.0, scalar2=None,
                            op0=mybir.AluOpType.max)
                        # seg (transposed, invc folded into S)
                        if par == 1 or last:
                            for km in range(KM):
                                if par == 1:  # full pair -> DoubleRow
                                    nc.tensor.matmul(
                                        out=node_ps[:, km, :],
                                        lhsT=m1sb[:, :, km * 128:(km + 1) * 128],
                                        rhs=S_t[:, t - 1:t + 1, :],
                                        start=first_seg, stop=last,
                                        perf_mode=DR, skip_group_check=True)
                                else:  # odd leftover single subtile
                                    nc.tensor.matmul(
                                        out=node_ps[:, km, :],
                                        lhsT=m1sb[:, 0, km * 128:(km + 1) * 128],
                                        rhs=S_t[:, t, :],
                                        start=first_seg, stop=True,
                                        skip_group_check=True)
                            first_seg = False
                        if last:
                            # S1T -> fp8 (split across engines)
                            s1 = spool.tile([128, KM, 128], F8, tag="s1", bufs=4)
                            nc.any.tensor_copy(out=s1[:],
                                               in_=node_ps[:, 0:KM, :])
                            # m2 node-major: h_own row block
                            m2c = ppool.tile([128, 2, 128], F32, tag="m2", bufs=1)
                            m2p = m2c[:, 0:F_out // 128, :]
                            for kk in range(KM // 2):
                                nc.tensor.matmul(
                                    out=m2p, lhsT=s1[:, 2 * kk:2 * kk + 2, :],
                                    rhs=w2_t[li][:, 2 * kk:2 * kk + 2, :],
                                    start=(kk == 0), stop=False, perf_mode=DR,
                                    skip_group_check=True)
                            nc.tensor.matmul(
                                out=m2p, lhsT=ones_t[:],
                                rhs=cbrow_t[li][:], start=False, stop=True,
                                skip_group_check=True)
                            if li < 2:
                                hsb = spool.tile([128, F_out], F8, tag="hsb")
                                nc.any.tensor_scalar(
                                    out=hsb[:], in0=m2p, scalar1=0.0,
                                    scalar2=None, op0=mybir.AluOpType.max)
                                nc.sync.dma_start(
                                    out=h_own[li + 1][w * 128:(w + 1) * 128, :]
                                    .bitcast(F8),
                                    in_=hsb[:])
                                # m2 transposed: hT for next layer, bias=cb col
                                for ho in range(2):
                                    pt = ppool.tile([128, 128], F32, tag="tp", bufs=1)
                                    for kk in range(KM // 2):
                                        nc.tensor.matmul(
                                            out=pt[:],
                                            lhsT=w2_t[li][:, 2 * kk:2 * kk + 2,
                                                          ho * 128:(ho + 1) * 128],
                                            rhs=s1[:, 2 * kk:2 * kk + 2, :],
                                            start=(kk == 0),
                                            stop=(kk == KM // 2 - 1),
                                            perf_mode=DR, skip_group_check=True)
                                    nc.any.tensor_scalar(
                                        out=hT_t[li + 1][:, w, ho, :],
                                        in0=pt[:],
                                        scalar1=cbcol_t[li][:, ho:ho + 1],
                                        scalar2=0.0,
                                        op0=mybir.AluOpType.add,
                                        op1=mybir.AluOpType.max)
                            else:
                                hsb = spool.tile([128, F_out], BF16, tag="hsb")
                                nc.scalar.activation(out=hsb[:], in_=m2p,
                                                     func=RELU)
                                nc.sync.dma_start(
                                    out=h_own3[w * 128:(w + 1) * 128, :],
                                    in_=hsb[:])
                    if skip_compute:
                        continue
                    if li < 2:
                        allgather(h_own[li + 1], h_full[li + 1])
                    else:
                        allgather(h_own3, h_full3)

                # ---------- pooling + FC + sigmoid ----------
                if debug_stage < 5 or skip_compute:
                    return
                F_last = plan.layer_dims[-1][2]
                hp = spool.tile([128, TP, F_last], BF16, tag="hp")
                for p0 in range(0, TP, 4):
                    pn = min(4, TP - p0)
                    nc.gpsimd.dma_gather(
                        hp[:, p0:p0 + pn, :], h_full3[:, :],
                        pool_idx[:, p0 * 8:(p0 + pn) * 8],
                        pn * 128, pn * 128, F_last, transpose=False,
                        queue_num=(p0 // 4) % 4)
                pool_c = ppool.tile([128, 2, 128], F32, tag="m2", bufs=1)
                pool_ps = pool_c[0:GPC, 0, :]
                for t in range(TP):
                    nc.tensor.matmul(out=pool_ps,
                                     lhsT=S2_t[:, t * GPC:(t + 1) * GPC],
                                     rhs=hp[:, t, :], start=(t == 0),
                                     stop=(t == TP - 1))
                pooled_sb = spool.tile([GPC, F_last], BF16, tag="pooled")
                nc.any.tensor_scalar(out=pooled_sb[:], in0=pool_ps,
                                     scalar1=ginv_t[:], scalar2=None,
                                     op0=mybir.AluOpType.mult)
                ptr_sb = spool.tile([F_last, GPC], BF16, tag="ptrsb")
                for bi in range(GPC // 32):
                    for bj in range(F_last // 32):
                        nc.vector.transpose(
                            out=ptr_sb[32 * bj:32 * bj + 32, 32 * bi:32 * bi + 32],
                            in_=pooled_sb[32 * bi:32 * bi + 32,
                                          32 * bj:32 * bj + 32])
                fc_c = ppool.tile([128, 128], F32, tag="tp", bufs=1)
                fc_ps = fc_c[0:GPC, 0:F_FC]
                nc.tensor.matmul(out=fc_ps, lhsT=ptr_sb[:], rhs=fc_wT_t[:],
                                 start=True, stop=True)
                logit = spool.tile([GPC, F_FC], F32, tag="logit")
                nc.vector.tensor_tensor(out=logit[:], in0=fc_ps, in1=fcb_t[:],
                                        op=mybir.AluOpType.add)
                nc.sync.dma_start(out=out_part[:], in_=logit[:])

            for _r in range(repeats):
                _body()

    nc.compile()
    return nc


_CACHE = {}


def run(inputs, G=G_REAL):
    plan = Plan(inputs, G)
    key = (plan.N, plan.G, plan.TP, tuple(plan.T_w))
    if key not in _CACHE:
        _CACHE[key] = build_program(plan)
    nc = _CACHE[key]
    res = bass_utils.run_bass_kernel_spmd(nc, plan.in_maps(),
                                          core_ids=list(range(N_CORES)))
    logits = np.concatenate([res.results[k]["out_part"] for k in range(N_CORES)], 0)
    out = 1.0 / (1.0 + np.exp(-logits.astype(np.float64)))
    return np.ascontiguousarray(out.astype(np.float32))


def kernel(**inputs) -> np.ndarray:
    return run(inputs, G=G_REAL)
